# revision 4
# baseline (speedup 1.0000x reference)
"""Trainium2 Bass kernel for nn_CMFA (dense_transformer, seq_len=1 cross-attention).

Math notes (exact simplifications vs the reference):
  - softmax over a single key is exactly 1.0, so mha(q,k,v) = lin(lin(v)); the
    q/k projections never influence the output.
  - Wv -> Wo -> fi2 is a linear chain, folded on the host:
      V = v1 @ A.T + i_ @ F.T + bcat,  A = fi2 @ (Wo @ Wv), F = fi2.

Precision plan (validated numerically, rel err ~6e-3 vs 2e-2 gate):
  - Dominant path (i -> fi1 -> i_ -> F -> out) in bf16: inputs, fi1/ft1
    weights, i_/t_ activations, F weights, output all bf16.
  - Attenuated path (v1/v2: A is ~5x smaller than F) in fp8 e4m3 with
    DoubleRow matmuls (2 MACs/cell/cycle): ci1, ct1 and the v-halves of V/T.
  - PSUM mixing: the fp8 half of V/T lands scaled by s_v*s_A, so the bf16
    F weights are pre-scaled by the same factor; one DVE op descales + bias.

Device layout: activations feature-major [feat, batch]; batch tiles of 512.
Pure data parallel across 8 cores; weights replicated.

DMA strategy: one dma_start's packets spread across all 16 DMA engines, so
transfer time is small and the ~600ns trigger on the issuing engine is the
real cost. Hence few, big DMAs: one per input x-tile (2MB via AP rearrange),
one per output half-tile. Engine split per tile: PE 132 matmuls, scalar 16
activations, DVE 8 fp8 copies + 8 scaled output writes, sync all DMA triggers.
"""

import numpy as np
import ml_dtypes

B, IMG, TAB, HID = 32768, 2048, 128, 512
NCORES = 8
BS = B // NCORES  # rows per core
NT = 512          # batch-tile (matmul moving/free dim)

# fp8 activation scales (powers of 2; absmax*scale ~ 90..160, fp8e4 max 240)
S_I8 = 16.0
S_T8 = 64.0
S_V1 = 64.0
S_V2 = 256.0

_CACHE = {}


def _pow2(x: float) -> float:
    return float(2.0 ** np.floor(np.log2(x)))


def _pack_blocks(WT: np.ndarray, K: int, M: int, dtype) -> np.ndarray:
    """[K*128, M*128] -> [128, K*M*128] with col ((k*M+m)*128 + j) = WT[k*128+p, m*128+j]."""
    out = WT.reshape(K, 128, M, 128).transpose(1, 0, 2, 3).reshape(128, K * M * 128)
    return np.ascontiguousarray(out.astype(dtype))


def _build_nc(bs: int, g_v1: float, g_v2: float, inv_sv: float, inv_st: float):
    import concourse.bass as bass
    import concourse.tile as tile
    from concourse import bacc, mybir

    f32 = mybir.dt.float32
    bf16 = mybir.dt.bfloat16
    f8 = mybir.dt.float8e4
    Relu = mybir.ActivationFunctionType.Relu
    DR = mybir.MatmulPerfMode.DoubleRow
    Mult = mybir.AluOpType.mult
    Add = mybir.AluOpType.add
    ntiles = bs // NT

    nc = bacc.Bacc("TRN2", target_bir_lowering=False, debug=False)

    iT_d = nc.dram_tensor("iT", [IMG, bs], bf16, kind="ExternalInput").ap()
    tT_d = nc.dram_tensor("tT", [TAB, bs], bf16, kind="ExternalInput").ap()
    w_fi1_d = nc.dram_tensor("w_fi1", [128, 64 * 128], bf16, kind="ExternalInput").ap()
    w_ft1_d = nc.dram_tensor("w_ft1", [128, 4 * 128], bf16, kind="ExternalInput").ap()
    w_ci1_d = nc.dram_tensor("w_ci1", [128, 16 * 128], f8, kind="ExternalInput").ap()
    w_ct1_d = nc.dram_tensor("w_ct1", [128, 16 * 128], f8, kind="ExternalInput").ap()
    w_AV_d = nc.dram_tensor("w_AV", [128, 16 * 128], f8, kind="ExternalInput").ap()
    w_AT_d = nc.dram_tensor("w_AT", [128, 16 * 128], f8, kind="ExternalInput").ap()
    w_FV_d = nc.dram_tensor("w_FV", [128, 16 * 128], bf16, kind="ExternalInput").ap()
    w_FT_d = nc.dram_tensor("w_FT", [128, 16 * 128], bf16, kind="ExternalInput").ap()
    bias_d = nc.dram_tensor("bias", [128, 24], f32, kind="ExternalInput").ap()
    out_d = nc.dram_tensor("outT", [2 * HID, bs], bf16, kind="ExternalOutput").ap()

    def dram_x(c0, r0=0, r1=IMG):
        return iT_d[r0:r1, c0:c0 + NT].rearrange("(c p) n -> p c n", p=128)

    def dram_out(half, c0):
        return out_d[512 * half:512 * (half + 1), c0:c0 + NT].rearrange(
            "(m p) n -> p m n", p=128)

    with tile.TileContext(nc) as tc:
        with (
            tc.tile_pool(name="w", bufs=1) as wpool,
            tc.tile_pool(name="x", bufs=2) as xpool,
            tc.tile_pool(name="h", bufs=2) as hpool,
            tc.tile_pool(name="o", bufs=2) as opool,
            tc.tile_pool(name="ps", bufs=8, space="PSUM") as pspool,
        ):
            wf1 = wpool.tile([128, 16, 4 * 128], bf16, name="w_fi1_t")
            wt1 = wpool.tile([128, 4 * 128], bf16, name="w_ft1_t")
            wci = wpool.tile([128, 4, 4 * 128], f8, name="w_ci1_t")
            wct = wpool.tile([128, 4, 4 * 128], f8, name="w_ct1_t")
            wAV = wpool.tile([128, 4, 4 * 128], f8, name="w_AV_t")
            wAT = wpool.tile([128, 4, 4 * 128], f8, name="w_AT_t")
            wFV = wpool.tile([128, 4, 4 * 128], bf16, name="w_FV_t")
            wFT = wpool.tile([128, 4, 4 * 128], bf16, name="w_FT_t")
            bt = wpool.tile([128, 24], f32, name="bias_t")

            # ---- preamble: big DMAs in consumption order, split across the
            # two HWDGE trigger engines (sync, scalar).
            xt_cur = xpool.tile([128, NT], bf16, tag="xt", bufs=2, name="xt_0")
            nc.sync.dma_start(xt_cur[:], tT_d[:, 0:NT])
            nc.sync.dma_start(wt1[:], w_ft1_d[:])
            x_cur = xpool.tile([128, 16, NT], bf16, tag="x", name="x3_0")
            for q in range(4):
                nc.sync.dma_start(x_cur[:, 4 * q:4 * (q + 1), :],
                                  dram_x(0, 512 * q, 512 * (q + 1)))
            nc.sync.dma_start(wct[:], w_ct1_d[:].rearrange("p (c n) -> p c n", c=4))
            nc.sync.dma_start(wci[:], w_ci1_d[:].rearrange("p (c n) -> p c n", c=4))

            nc.scalar.dma_start(bt[:], bias_d[:])
            for q in range(4):
                nc.scalar.dma_start(
                    wf1[:, 4 * q:4 * (q + 1), :],
                    w_fi1_d[:, 2048 * q:2048 * (q + 1)].rearrange(
                        "p (c n) -> p c n", c=4))
            nc.scalar.dma_start(wFV[:], w_FV_d[:].rearrange("p (c n) -> p c n", c=4))
            nc.scalar.dma_start(wAV[:], w_AV_d[:].rearrange("p (c n) -> p c n", c=4))
            nc.scalar.dma_start(wFT[:], w_FT_d[:].rearrange("p (c n) -> p c n", c=4))
            nc.scalar.dma_start(wAT[:], w_AT_d[:].rearrange("p (c n) -> p c n", c=4))

            def mm_dr(ps_ap, wtile3, kp, m, mov3, start, stop):
                nc.tensor.matmul(
                    ps_ap,
                    wtile3[:, 2 * kp:2 * kp + 2, m * 128:(m + 1) * 128],
                    mov3[:, 2 * kp:2 * kp + 2, :],
                    start=start, stop=stop, perf_mode=DR,
                )

            for n in range(ntiles):
                c0 = n * NT

                # ---- t_ = relu(t @ ft1.T + b): bf16 ----
                ps2 = [pspool.tile([128, NT], f32, tag="ps", name=f"ps2_{n}_{m}")
                       for m in range(4)]
                for m in range(4):
                    nc.tensor.matmul(ps2[m][:], wt1[:, m * 128:(m + 1) * 128],
                                     xt_cur[:], start=True, stop=True)
                t_b = [hpool.tile([128, NT], bf16, tag="t_", bufs=8,
                                  name=f"t_b_{n}_{m}") for m in range(4)]
                t_8 = hpool.tile([128, 4, NT], f8, tag="t8", name=f"t_8_{n}")
                for m in range(4):
                    nc.scalar.activation(t_b[m][:], ps2[m][:], Relu,
                                         bias=bt[:, 4 + m:5 + m])
                for m in range(4):
                    nc.vector.tensor_scalar_mul(t_8[:, m, :], t_b[m][:], S_T8)

                # ---- i_ = relu(i @ fi1.T + b): bf16 ----
                ps1 = [pspool.tile([128, NT], f32, tag="ps", name=f"ps1_{n}_{m}")
                       for m in range(4)]
                for k in range(16):
                    for m in range(4):
                        nc.tensor.matmul(ps1[m][:],
                                         wf1[:, k, m * 128:(m + 1) * 128],
                                         x_cur[:, k, :],
                                         start=k == 0, stop=k == 15)

                # prefetch next tile's inputs (single big DMAs)
                if n + 1 < ntiles:
                    x_nxt = xpool.tile([128, 16, NT], bf16, tag="x",
                                       name=f"x3_{n + 1}")
                    nc.sync.dma_start(x_nxt[:], dram_x(c0 + NT))
                    xt_nxt = xpool.tile([128, NT], bf16, tag="xt", bufs=2,
                                        name=f"xt_{n + 1}")
                    nc.sync.dma_start(xt_nxt[:], tT_d[:, c0 + NT:c0 + 2 * NT])

                i_b = [hpool.tile([128, NT], bf16, tag="i_", bufs=8,
                                  name=f"i_b_{n}_{m}") for m in range(4)]
                i_8 = hpool.tile([128, 4, NT], f8, tag="i8", name=f"i_8_{n}")
                for m in range(4):
                    nc.scalar.activation(i_b[m][:], ps1[m][:], Relu,
                                         bias=bt[:, m:m + 1])
                for m in range(4):
                    nc.vector.tensor_scalar_mul(i_8[:, m, :], i_b[m][:], S_I8)

                # ---- v2 = relu(t_ @ ct1.T + b): fp8 DoubleRow ----
                ps4 = [pspool.tile([128, NT], f32, tag="ps", name=f"ps4_{n}_{m}")
                       for m in range(4)]
                for kp in range(2):
                    for m in range(4):
                        mm_dr(ps4[m][:], wct, kp, m, t_8, kp == 0, kp == 1)
                v2_8 = hpool.tile([128, 4, NT], f8, tag="v2", name=f"v2_8_{n}")
                for m in range(4):
                    nc.scalar.activation(v2_8[:, m, :], ps4[m][:], Relu,
                                         bias=bt[:, 12 + m:13 + m], scale=g_v2)

                # ---- v1 = relu(i_ @ ci1.T + b): fp8 DoubleRow ----
                ps3 = [pspool.tile([128, NT], f32, tag="ps", name=f"ps3_{n}_{m}")
                       for m in range(4)]
                for kp in range(2):
                    for m in range(4):
                        mm_dr(ps3[m][:], wci, kp, m, i_8, kp == 0, kp == 1)
                v1_8 = hpool.tile([128, 4, NT], f8, tag="v1", name=f"v1_8_{n}")
                for m in range(4):
                    nc.scalar.activation(v1_8[:, m, :], ps3[m][:], Relu,
                                         bias=bt[:, 8 + m:9 + m], scale=g_v1)

                # ---- V = (v1 @ A.T)*sv + i_ @ (F*sv).T, then descale ----
                psV = [pspool.tile([128, NT], f32, tag="ps", name=f"psV_{n}_{m}")
                       for m in range(4)]
                for k in range(4):
                    for m in range(4):
                        nc.tensor.matmul(psV[m][:],
                                         wFV[:, k, m * 128:(m + 1) * 128],
                                         i_b[k][:], start=k == 0, stop=False)
                for kp in range(2):
                    for m in range(4):
                        mm_dr(psV[m][:], wAV, kp, m, v1_8, False, kp == 1)
                oV = opool.tile([128, 4, NT], bf16, tag="oV", name=f"oV_{n}")
                for m in range(4):
                    nc.vector.tensor_scalar(oV[:, m, :], psV[m][:], inv_sv,
                                            bt[:, 16 + m:17 + m], Mult, Add)
                nc.sync.dma_start(dram_out(0, c0), oV[:])

                # ---- T = (v2 @ A.T)*st + t_ @ (F*st).T, then descale ----
                psT = [pspool.tile([128, NT], f32, tag="ps", name=f"psT_{n}_{m}")
                       for m in range(4)]
                for k in range(4):
                    for m in range(4):
                        nc.tensor.matmul(psT[m][:],
                                         wFT[:, k, m * 128:(m + 1) * 128],
                                         t_b[k][:], start=k == 0, stop=False)
                for kp in range(2):
                    for m in range(4):
                        mm_dr(psT[m][:], wAT, kp, m, v2_8, False, kp == 1)
                if n + 1 == ntiles:
                    # drain the last tile m-block by m-block for a short tail
                    for m in range(4):
                        oT = opool.tile([128, NT], bf16, tag="oTl",
                                        bufs=4, name=f"oT_{n}_{m}")
                        nc.vector.tensor_scalar(oT[:], psT[m][:], inv_st,
                                                bt[:, 20 + m:21 + m], Mult, Add)
                        eng = nc.scalar if m % 2 else nc.sync
                        eng.dma_start(
                            out_d[512 + 128 * m:512 + 128 * (m + 1), c0:c0 + NT],
                            oT[:])
                else:
                    oT = opool.tile([128, 4, NT], bf16, tag="oT", name=f"oT_{n}")
                    for m in range(4):
                        nc.vector.tensor_scalar(oT[:, m, :], psT[m][:], inv_st,
                                                bt[:, 20 + m:21 + m], Mult, Add)
                    nc.sync.dma_start(dram_out(1, c0), oT[:])

                if n + 1 < ntiles:
                    x_cur = x_nxt
                    xt_cur = xt_nxt

    nc.compile()
    return nc


def _host_pack(inp: dict):
    f8d = np.float64
    bf = ml_dtypes.bfloat16
    e4 = ml_dtypes.float8_e4m3

    def fold(wv, bv, wo, bo, f_w, f_b):
        Wvo = wo.astype(f8d) @ wv.astype(f8d)
        bvo = wo.astype(f8d) @ bv.astype(f8d) + bo.astype(f8d)
        A = (f_w.astype(f8d) @ Wvo).astype(np.float32)
        F = f_w.astype(np.float32)
        bcat = (f_w.astype(f8d) @ bvo + f_b.astype(f8d)).astype(np.float32)
        return A, F, bcat

    AV, FV, bcatV = fold(inp["aV_wv"], inp["aV_bv"], inp["aV_wo"], inp["aV_bo"],
                         inp["fi2_w"], inp["fi2_b"])
    AT, FT, bcatT = fold(inp["aT_wv"], inp["aT_bv"], inp["aT_wo"], inp["aT_bo"],
                         inp["ft2_w"], inp["ft2_b"])

    s_wci = _pow2(160.0 / float(np.abs(inp["ci1_w"]).max()))
    s_wct = _pow2(160.0 / float(np.abs(inp["ct1_w"]).max()))
    s_AV = _pow2(160.0 / float(np.abs(AV).max()))
    s_AT = _pow2(160.0 / float(np.abs(AT).max()))
    sv = np.float32(S_V1 * s_AV)
    st = np.float32(S_V2 * s_AT)

    def q8(x, s):
        return np.clip(x * np.float32(s), -240, 240)

    tr = lambda w: np.ascontiguousarray(w.T)
    weights = {
        "w_fi1": _pack_blocks(tr(inp["fi1_w"]).astype(np.float32), 16, 4, bf),
        "w_ft1": _pack_blocks(tr(inp["ft1_w"]).astype(np.float32), 1, 4, bf),
        "w_ci1": _pack_blocks(q8(tr(inp["ci1_w"]), s_wci), 4, 4, e4),
        "w_ct1": _pack_blocks(q8(tr(inp["ct1_w"]), s_wct), 4, 4, e4),
        "w_AV": _pack_blocks(q8(tr(AV), s_AV), 4, 4, e4),
        "w_AT": _pack_blocks(q8(tr(AT), s_AT), 4, 4, e4),
        "w_FV": _pack_blocks(tr(FV) * sv, 4, 4, bf),
        "w_FT": _pack_blocks(tr(FT) * st, 4, 4, bf),
    }
    cols = []
    for b in (inp["fi1_b"], inp["ft1_b"],
              np.float32(S_V1) * inp["ci1_b"], np.float32(S_V2) * inp["ct1_b"],
              bcatV, bcatT):
        b = np.asarray(b, dtype=np.float32)
        for m in range(4):
            cols.append(b[128 * m:128 * (m + 1)])
    weights["bias"] = np.ascontiguousarray(np.stack(cols, axis=1),
                                           dtype=np.float32)
    scales = dict(
        g_v1=float(S_V1 / (S_I8 * s_wci)),
        g_v2=float(S_V2 / (S_T8 * s_wct)),
        inv_sv=float(1.0 / sv),
        inv_st=float(1.0 / st),
    )
    return weights, scales


def kernel(**inputs) -> np.ndarray:
    from concourse import bass_utils

    i = np.asarray(inputs["i"], dtype=np.float32)
    t = np.asarray(inputs["t"], dtype=np.float32)
    weights, scales = _host_pack(inputs)

    if "nc" not in _CACHE:
        _CACHE["nc"] = _build_nc(BS, **scales)
    nc = _CACHE["nc"]

    in_maps = []
    for c in range(NCORES):
        sl = slice(c * BS, (c + 1) * BS)
        m = dict(weights)
        m["iT"] = np.ascontiguousarray(i[sl].T.astype(ml_dtypes.bfloat16))
        m["tT"] = np.ascontiguousarray(t[sl].T.astype(ml_dtypes.bfloat16))
        in_maps.append(m)

    res = bass_utils.run_bass_kernel_spmd(nc, in_maps, core_ids=list(range(NCORES)))

    out = np.empty((B, 2 * HID), dtype=np.float32)
    for c in range(NCORES):
        out[c * BS:(c + 1) * BS] = res.results[c]["outT"].astype(np.float32).T
    return out


# revision 5
# speedup vs baseline: 1.1742x; 1.1742x over previous
"""Trainium2 Bass kernel for nn_CMFA (dense_transformer, seq_len=1 cross-attention).

Math notes (exact simplifications vs the reference):
  - softmax over a single key is exactly 1.0, so mha(q,k,v) = lin(lin(v)); the
    q/k projections never influence the output.
  - Wv -> Wo -> fi2 is a linear chain, folded on the host:
      V = v1 @ A.T + i_ @ F.T + bcat,  A = fi2 @ (Wo @ Wv), F = fi2.

Precision plan (validated numerically, rel err ~6e-3 vs 2e-2 gate):
  - Dominant path (i -> fi1 -> i_ -> F -> out) in bf16: inputs, fi1/ft1
    weights, i_/t_ activations, F weights, output all bf16.
  - Attenuated path (v1/v2: A is ~5x smaller than F) in fp8 e4m3 with
    DoubleRow matmuls (2 MACs/cell/cycle): ci1, ct1 and the v-halves of V/T.
  - PSUM mixing: the fp8 half of V/T lands scaled by s_v*s_A, so the bf16
    F weights are pre-scaled by the same factor; one DVE op descales + bias.

Device layout: activations feature-major [feat, batch]; batch tiles of 512.
Pure data parallel across 8 cores; weights replicated.

DMA strategy: one dma_start's packets spread across all 16 DMA engines, so
transfer time is small and the ~600ns trigger on the issuing engine is the
real cost. Hence few, big DMAs: one per input x-tile (2MB via AP rearrange),
one per output half-tile. Engine split per tile: PE 132 matmuls, scalar 16
activations, DVE 8 fp8 copies + 8 scaled output writes, sync all DMA triggers.
"""

import numpy as np
import ml_dtypes

B, IMG, TAB, HID = 32768, 2048, 128, 512
NCORES = 8
BS = B // NCORES  # rows per core
NT = 512          # batch-tile (matmul moving/free dim)

# fp8 activation scales (powers of 2; absmax*scale ~ 90..160, fp8e4 max 240)
S_I8 = 16.0
S_T8 = 64.0
S_V1 = 64.0
S_V2 = 256.0

_CACHE = {}


def _pow2(x: float) -> float:
    return float(2.0 ** np.floor(np.log2(x)))


def _pack_blocks(WT: np.ndarray, K: int, M: int, dtype) -> np.ndarray:
    """[K*128, M*128] -> [128, K*M*128] with col ((k*M+m)*128 + j) = WT[k*128+p, m*128+j]."""
    out = WT.reshape(K, 128, M, 128).transpose(1, 0, 2, 3).reshape(128, K * M * 128)
    return np.ascontiguousarray(out.astype(dtype))


def _build_nc(bs: int, g_v1: float, g_v2: float, inv_sv: float, inv_st: float):
    import concourse.bass as bass
    import concourse.tile as tile
    from concourse import bacc, mybir

    f32 = mybir.dt.float32
    bf16 = mybir.dt.bfloat16
    f8 = mybir.dt.float8e4
    Relu = mybir.ActivationFunctionType.Relu
    DR = mybir.MatmulPerfMode.DoubleRow
    Mult = mybir.AluOpType.mult
    Add = mybir.AluOpType.add
    ntiles = bs // NT

    nc = bacc.Bacc("TRN2", target_bir_lowering=False, debug=False)

    iT_d = nc.dram_tensor("iT", [IMG, bs], bf16, kind="ExternalInput").ap()
    tT_d = nc.dram_tensor("tT", [TAB, bs], bf16, kind="ExternalInput").ap()
    w_fi1_d = nc.dram_tensor("w_fi1", [128, 64 * 128], bf16, kind="ExternalInput").ap()
    w_ft1_d = nc.dram_tensor("w_ft1", [128, 4 * 128], bf16, kind="ExternalInput").ap()
    w_ci1_d = nc.dram_tensor("w_ci1", [128, 16 * 128], f8, kind="ExternalInput").ap()
    w_ct1_d = nc.dram_tensor("w_ct1", [128, 16 * 128], f8, kind="ExternalInput").ap()
    w_AV_d = nc.dram_tensor("w_AV", [128, 16 * 128], f8, kind="ExternalInput").ap()
    w_AT_d = nc.dram_tensor("w_AT", [128, 16 * 128], f8, kind="ExternalInput").ap()
    w_FV_d = nc.dram_tensor("w_FV", [128, 16 * 128], bf16, kind="ExternalInput").ap()
    w_FT_d = nc.dram_tensor("w_FT", [128, 16 * 128], bf16, kind="ExternalInput").ap()
    bias_d = nc.dram_tensor("bias", [128, 24], f32, kind="ExternalInput").ap()
    out_d = nc.dram_tensor("outT", [2 * HID, bs], bf16, kind="ExternalOutput").ap()

    def dram_x(c0, r0=0, r1=IMG):
        return iT_d[r0:r1, c0:c0 + NT].rearrange("(c p) n -> p c n", p=128)

    def dram_out(half, c0):
        return out_d[512 * half:512 * (half + 1), c0:c0 + NT].rearrange(
            "(m p) n -> p m n", p=128)

    with tile.TileContext(nc) as tc:
        with (
            tc.tile_pool(name="w", bufs=1) as wpool,
            tc.tile_pool(name="x", bufs=2) as xpool,
            tc.tile_pool(name="h", bufs=2) as hpool,
            tc.tile_pool(name="o", bufs=2) as opool,
            tc.tile_pool(name="ps", bufs=8, space="PSUM") as pspool,
        ):
            wf1 = wpool.tile([128, 16 * 512], bf16, name="w_fi1_t")
            wt1 = wpool.tile([128, 4 * 128], bf16, name="w_ft1_t")
            wci = wpool.tile([128, 4, 4 * 128], f8, name="w_ci1_t")
            wct = wpool.tile([128, 4, 4 * 128], f8, name="w_ct1_t")
            wAV = wpool.tile([128, 4, 4 * 128], f8, name="w_AV_t")
            wAT = wpool.tile([128, 4, 4 * 128], f8, name="w_AT_t")
            wFV = wpool.tile([128, 4 * 512], bf16, name="w_FV_t")
            wFT = wpool.tile([128, 4 * 512], bf16, name="w_FT_t")
            bt = wpool.tile([128, 24], f32, name="bias_t")

            # ---- preamble: big DMAs in consumption order, split across the
            # two HWDGE trigger engines (sync, scalar).
            xt_cur = xpool.tile([128, NT], bf16, tag="xt", bufs=2, name="xt_0")
            nc.sync.dma_start(xt_cur[:], tT_d[:, 0:NT])
            nc.sync.dma_start(wt1[:], w_ft1_d[:])
            x_cur = xpool.tile([128, 16 * NT], bf16, tag="x", name="x3_0")
            for q in range(4):
                nc.sync.dma_start(x_cur[:, 4 * q * NT:4 * (q + 1) * NT],
                                  dram_x(0, 512 * q, 512 * (q + 1)))
            nc.sync.dma_start(wct[:], w_ct1_d[:].rearrange("p (c n) -> p c n", c=4))
            nc.sync.dma_start(wci[:], w_ci1_d[:].rearrange("p (c n) -> p c n", c=4))

            nc.scalar.dma_start(bt[:], bias_d[:])
            for q in range(4):
                nc.scalar.dma_start(
                    wf1[:, 2048 * q:2048 * (q + 1)],
                    w_fi1_d[:, 2048 * q:2048 * (q + 1)])
            nc.scalar.dma_start(wFV[:], w_FV_d[:])
            nc.scalar.dma_start(wAV[:], w_AV_d[:].rearrange("p (c n) -> p c n", c=4))
            nc.scalar.dma_start(wFT[:], w_FT_d[:])
            nc.scalar.dma_start(wAT[:], w_AT_d[:].rearrange("p (c n) -> p c n", c=4))

            def mm_dr(ps_ap, wtile3, kp, m, mov3, start, stop):
                nc.tensor.matmul(
                    ps_ap,
                    wtile3[:, 2 * kp:2 * kp + 2, m * 128:(m + 1) * 128],
                    mov3[:, 2 * kp:2 * kp + 2, :],
                    start=start, stop=stop, perf_mode=DR,
                )

            for n in range(ntiles):
                c0 = n * NT

                # ---- t_ = relu(t @ ft1.T + b): bf16 ----
                ps2 = [pspool.tile([128, NT], f32, tag="ps", name=f"ps2_{n}_{m}")
                       for m in range(4)]
                for m in range(4):
                    nc.tensor.matmul(ps2[m][:], wt1[:, m * 128:(m + 1) * 128],
                                     xt_cur[:], start=True, stop=True)
                t_b = [hpool.tile([128, NT], bf16, tag="t_", bufs=8,
                                  name=f"t_b_{n}_{m}") for m in range(4)]
                t_8 = hpool.tile([128, 4, NT], f8, tag="t8", name=f"t_8_{n}")
                for m in range(4):
                    nc.scalar.activation(t_b[m][:], ps2[m][:], Relu,
                                         bias=bt[:, 4 + m:5 + m])
                for m in range(4):
                    nc.vector.tensor_scalar_mul(t_8[:, m, :], t_b[m][:], S_T8)

                # ---- i_ = relu(i @ fi1.T + b): bf16 ----
                ps1 = [pspool.tile([128, NT], f32, tag="ps", name=f"ps1_{n}_{m}")
                       for m in range(4)]
                for k in range(16):
                    for m in range(4):
                        nc.tensor.matmul(
                            ps1[m][:],
                            wf1[:, 512 * k + 128 * m:512 * k + 128 * (m + 1)],
                            x_cur[:, NT * k:NT * (k + 1)],
                            start=k == 0, stop=k == 15)

                # prefetch next tile's inputs (single big DMAs)
                if n + 1 < ntiles:
                    x_nxt = xpool.tile([128, 16 * NT], bf16, tag="x",
                                       name=f"x3_{n + 1}")
                    nc.sync.dma_start(x_nxt[:], dram_x(c0 + NT))
                    xt_nxt = xpool.tile([128, NT], bf16, tag="xt", bufs=2,
                                        name=f"xt_{n + 1}")
                    nc.sync.dma_start(xt_nxt[:], tT_d[:, c0 + NT:c0 + 2 * NT])

                i_b = [hpool.tile([128, NT], bf16, tag="i_", bufs=8,
                                  name=f"i_b_{n}_{m}") for m in range(4)]
                i_8 = hpool.tile([128, 4, NT], f8, tag="i8", name=f"i_8_{n}")
                for m in range(4):
                    nc.scalar.activation(i_b[m][:], ps1[m][:], Relu,
                                         bias=bt[:, m:m + 1])
                for m in range(4):
                    nc.vector.tensor_scalar_mul(i_8[:, m, :], i_b[m][:], S_I8)

                # ---- v2 = relu(t_ @ ct1.T + b): fp8 DoubleRow ----
                ps4 = [pspool.tile([128, NT], f32, tag="ps", name=f"ps4_{n}_{m}")
                       for m in range(4)]
                for kp in range(2):
                    for m in range(4):
                        mm_dr(ps4[m][:], wct, kp, m, t_8, kp == 0, kp == 1)
                v2_8 = hpool.tile([128, 4, NT], f8, tag="v2", name=f"v2_8_{n}")
                for m in range(4):
                    nc.scalar.activation(v2_8[:, m, :], ps4[m][:], Relu,
                                         bias=bt[:, 12 + m:13 + m], scale=g_v2)

                # ---- v1 = relu(i_ @ ci1.T + b): fp8 DoubleRow ----
                ps3 = [pspool.tile([128, NT], f32, tag="ps", name=f"ps3_{n}_{m}")
                       for m in range(4)]
                for kp in range(2):
                    for m in range(4):
                        mm_dr(ps3[m][:], wci, kp, m, i_8, kp == 0, kp == 1)
                v1_8 = hpool.tile([128, 4, NT], f8, tag="v1", name=f"v1_8_{n}")
                for m in range(4):
                    nc.scalar.activation(v1_8[:, m, :], ps3[m][:], Relu,
                                         bias=bt[:, 8 + m:9 + m], scale=g_v1)

                # ---- V = (v1 @ A.T)*sv + i_ @ (F*sv).T, then descale ----
                psV = [pspool.tile([128, NT], f32, tag="ps", name=f"psV_{n}_{m}")
                       for m in range(4)]
                for k in range(4):
                    for m in range(4):
                        nc.tensor.matmul(
                            psV[m][:],
                            wFV[:, 512 * k + 128 * m:512 * k + 128 * (m + 1)],
                            i_b[k][:], start=k == 0, stop=False)
                for kp in range(2):
                    for m in range(4):
                        mm_dr(psV[m][:], wAV, kp, m, v1_8, False, kp == 1)
                oV = opool.tile([128, 4, NT], bf16, tag="oV", name=f"oV_{n}")
                for m in range(4):
                    nc.vector.tensor_scalar(oV[:, m, :], psV[m][:], inv_sv,
                                            bt[:, 16 + m:17 + m], Mult, Add)
                nc.sync.dma_start(dram_out(0, c0), oV[:])

                # ---- T = (v2 @ A.T)*st + t_ @ (F*st).T, then descale ----
                psT = [pspool.tile([128, NT], f32, tag="ps", name=f"psT_{n}_{m}")
                       for m in range(4)]
                for k in range(4):
                    for m in range(4):
                        nc.tensor.matmul(
                            psT[m][:],
                            wFT[:, 512 * k + 128 * m:512 * k + 128 * (m + 1)],
                            t_b[k][:], start=k == 0, stop=False)
                for kp in range(2):
                    for m in range(4):
                        mm_dr(psT[m][:], wAT, kp, m, v2_8, False, kp == 1)
                if n + 1 == ntiles:
                    # drain the last tile m-block by m-block for a short tail
                    for m in range(4):
                        oT = opool.tile([128, NT], bf16, tag="oTl",
                                        bufs=4, name=f"oT_{n}_{m}")
                        nc.vector.tensor_scalar(oT[:], psT[m][:], inv_st,
                                                bt[:, 20 + m:21 + m], Mult, Add)
                        eng = nc.scalar if m % 2 else nc.sync
                        eng.dma_start(
                            out_d[512 + 128 * m:512 + 128 * (m + 1), c0:c0 + NT],
                            oT[:])
                else:
                    oT = opool.tile([128, 4, NT], bf16, tag="oT", name=f"oT_{n}")
                    for m in range(4):
                        nc.vector.tensor_scalar(oT[:, m, :], psT[m][:], inv_st,
                                                bt[:, 20 + m:21 + m], Mult, Add)
                    nc.sync.dma_start(dram_out(1, c0), oT[:])

                if n + 1 < ntiles:
                    x_cur = x_nxt
                    xt_cur = xt_nxt

    nc.compile()
    return nc


def _host_pack(inp: dict):
    f8d = np.float64
    bf = ml_dtypes.bfloat16
    e4 = ml_dtypes.float8_e4m3

    def fold(wv, bv, wo, bo, f_w, f_b):
        Wvo = wo.astype(f8d) @ wv.astype(f8d)
        bvo = wo.astype(f8d) @ bv.astype(f8d) + bo.astype(f8d)
        A = (f_w.astype(f8d) @ Wvo).astype(np.float32)
        F = f_w.astype(np.float32)
        bcat = (f_w.astype(f8d) @ bvo + f_b.astype(f8d)).astype(np.float32)
        return A, F, bcat

    AV, FV, bcatV = fold(inp["aV_wv"], inp["aV_bv"], inp["aV_wo"], inp["aV_bo"],
                         inp["fi2_w"], inp["fi2_b"])
    AT, FT, bcatT = fold(inp["aT_wv"], inp["aT_bv"], inp["aT_wo"], inp["aT_bo"],
                         inp["ft2_w"], inp["ft2_b"])

    s_wci = _pow2(160.0 / float(np.abs(inp["ci1_w"]).max()))
    s_wct = _pow2(160.0 / float(np.abs(inp["ct1_w"]).max()))
    s_AV = _pow2(160.0 / float(np.abs(AV).max()))
    s_AT = _pow2(160.0 / float(np.abs(AT).max()))
    sv = np.float32(S_V1 * s_AV)
    st = np.float32(S_V2 * s_AT)

    def q8(x, s):
        return np.clip(x * np.float32(s), -240, 240)

    tr = lambda w: np.ascontiguousarray(w.T)
    weights = {
        "w_fi1": _pack_blocks(tr(inp["fi1_w"]).astype(np.float32), 16, 4, bf),
        "w_ft1": _pack_blocks(tr(inp["ft1_w"]).astype(np.float32), 1, 4, bf),
        "w_ci1": _pack_blocks(q8(tr(inp["ci1_w"]), s_wci), 4, 4, e4),
        "w_ct1": _pack_blocks(q8(tr(inp["ct1_w"]), s_wct), 4, 4, e4),
        "w_AV": _pack_blocks(q8(tr(AV), s_AV), 4, 4, e4),
        "w_AT": _pack_blocks(q8(tr(AT), s_AT), 4, 4, e4),
        "w_FV": _pack_blocks(tr(FV) * sv, 4, 4, bf),
        "w_FT": _pack_blocks(tr(FT) * st, 4, 4, bf),
    }
    cols = []
    for b in (inp["fi1_b"], inp["ft1_b"],
              np.float32(S_V1) * inp["ci1_b"], np.float32(S_V2) * inp["ct1_b"],
              bcatV, bcatT):
        b = np.asarray(b, dtype=np.float32)
        for m in range(4):
            cols.append(b[128 * m:128 * (m + 1)])
    weights["bias"] = np.ascontiguousarray(np.stack(cols, axis=1),
                                           dtype=np.float32)
    scales = dict(
        g_v1=float(S_V1 / (S_I8 * s_wci)),
        g_v2=float(S_V2 / (S_T8 * s_wct)),
        inv_sv=float(1.0 / sv),
        inv_st=float(1.0 / st),
    )
    return weights, scales


def kernel(**inputs) -> np.ndarray:
    from concourse import bass_utils

    i = np.asarray(inputs["i"], dtype=np.float32)
    t = np.asarray(inputs["t"], dtype=np.float32)
    weights, scales = _host_pack(inputs)

    if "nc" not in _CACHE:
        _CACHE["nc"] = _build_nc(BS, **scales)
    nc = _CACHE["nc"]

    in_maps = []
    for c in range(NCORES):
        sl = slice(c * BS, (c + 1) * BS)
        m = dict(weights)
        m["iT"] = np.ascontiguousarray(i[sl].T.astype(ml_dtypes.bfloat16))
        m["tT"] = np.ascontiguousarray(t[sl].T.astype(ml_dtypes.bfloat16))
        in_maps.append(m)

    res = bass_utils.run_bass_kernel_spmd(nc, in_maps, core_ids=list(range(NCORES)))

    out = np.empty((B, 2 * HID), dtype=np.float32)
    for c in range(NCORES):
        out[c * BS:(c + 1) * BS] = res.results[c]["outT"].astype(np.float32).T
    return out


# revision 9
# speedup vs baseline: 1.1832x; 1.0076x over previous
"""Trainium2 Bass kernel for nn_CMFA (dense_transformer, seq_len=1 cross-attention).

Math notes (exact simplifications vs the reference):
  - softmax over a single key is exactly 1.0, so mha(q,k,v) = lin(lin(v)); the
    q/k projections never influence the output.
  - Wv -> Wo -> fi2 is a linear chain, folded on the host:
      V = v1 @ A.T + i_ @ F.T + bcat,  A = fi2 @ (Wo @ Wv), F = fi2.

Precision plan (validated numerically, rel err ~6e-3 vs 2e-2 gate):
  - Dominant path (i -> fi1 -> i_ -> F -> out) in bf16: inputs, fi1/ft1
    weights, i_/t_ activations, F weights, output all bf16.
  - Attenuated path (v1/v2: A is ~5x smaller than F) in fp8 e4m3 with
    DoubleRow matmuls (2 MACs/cell/cycle): ci1, ct1 and the v-halves of V/T.
  - PSUM mixing: the fp8 half of V/T lands scaled by s_v*s_A, so the bf16
    F weights are pre-scaled by the same factor; one DVE op descales + bias.

Device layout: activations feature-major [feat, batch]; batch tiles of 512.
Pure data parallel across 8 cores; weights replicated.

DMA strategy: one dma_start's packets spread across all 16 DMA engines, so
transfer time is small and the ~600ns trigger on the issuing engine is the
real cost. Hence few, big DMAs: one per input x-tile (2MB via AP rearrange),
one per output half-tile. Engine split per tile: PE 132 matmuls, scalar 16
activations, DVE 8 fp8 copies + 8 scaled output writes, sync all DMA triggers.
"""

import numpy as np
import ml_dtypes

B, IMG, TAB, HID = 32768, 2048, 128, 512
NCORES = 8
BS = B // NCORES  # rows per core
NT = 512          # batch-tile (matmul moving/free dim)

# fp8 activation scales (powers of 2; absmax*scale ~ 90..160, fp8e4 max 240)
S_I8 = 16.0
S_T8 = 64.0
S_V1 = 64.0
S_V2 = 256.0

# DoubleRowSwInterleave: host pre-interleaves fp8 weights so LDWEIGHTS
# reads contiguously instead of paying the 256-col interleaved load.
SWI = False

_CACHE = {}


def _pow2(x: float) -> float:
    return float(2.0 ** np.floor(np.log2(x)))


def _pack_blocks(WT: np.ndarray, K: int, M: int, dtype) -> np.ndarray:
    """[K*128, M*128] -> [128, K*M*128] with col ((k*M+m)*128 + j) = WT[k*128+p, m*128+j]."""
    out = WT.reshape(K, 128, M, 128).transpose(1, 0, 2, 3).reshape(128, K * M * 128)
    return np.ascontiguousarray(out.astype(dtype))


def _build_nc(bs: int, g_v1: float, g_v2: float, inv_sv: float, inv_st: float):
    import concourse.bass as bass
    import concourse.tile as tile
    from concourse import bacc, mybir

    f32 = mybir.dt.float32
    bf16 = mybir.dt.bfloat16
    f8 = mybir.dt.float8e4
    Relu = mybir.ActivationFunctionType.Relu
    DR = (mybir.MatmulPerfMode.DoubleRowSwInterleave if SWI else
          mybir.MatmulPerfMode.DoubleRow)
    Mult = mybir.AluOpType.mult
    Add = mybir.AluOpType.add
    ntiles = bs // NT

    nc = bacc.Bacc("TRN2", target_bir_lowering=False, debug=False)

    iT_d = nc.dram_tensor("iT", [IMG, bs], bf16, kind="ExternalInput").ap()
    tT_d = nc.dram_tensor("tT", [TAB, bs], bf16, kind="ExternalInput").ap()
    w_fi1_d = nc.dram_tensor("w_fi1", [128, 64 * 128], bf16, kind="ExternalInput").ap()
    w_ft1_d = nc.dram_tensor("w_ft1", [128, 4 * 128], bf16, kind="ExternalInput").ap()
    w_ci1_d = nc.dram_tensor("w_ci1", [128, 16 * 128], f8, kind="ExternalInput").ap()
    w_ct1_d = nc.dram_tensor("w_ct1", [128, 16 * 128], f8, kind="ExternalInput").ap()
    w_AV_d = nc.dram_tensor("w_AV", [128, 16 * 128], f8, kind="ExternalInput").ap()
    w_AT_d = nc.dram_tensor("w_AT", [128, 16 * 128], f8, kind="ExternalInput").ap()
    w_FV_d = nc.dram_tensor("w_FV", [128, 16 * 128], bf16, kind="ExternalInput").ap()
    w_FT_d = nc.dram_tensor("w_FT", [128, 16 * 128], bf16, kind="ExternalInput").ap()
    bias_d = nc.dram_tensor("bias", [128, 24], f32, kind="ExternalInput").ap()
    out_d = nc.dram_tensor("outT", [2 * HID, bs], bf16, kind="ExternalOutput").ap()

    def dram_x(c0, r0=0, r1=IMG):
        return iT_d[r0:r1, c0:c0 + NT].rearrange("(c p) n -> p c n", p=128)

    def dram_out(half, c0):
        return out_d[512 * half:512 * (half + 1), c0:c0 + NT].rearrange(
            "(m p) n -> p m n", p=128)

    with tile.TileContext(nc) as tc:
        with (
            tc.tile_pool(name="w", bufs=1) as wpool,
            tc.tile_pool(name="x", bufs=2) as xpool,
            tc.tile_pool(name="h", bufs=2) as hpool,
            tc.tile_pool(name="o", bufs=2) as opool,
            tc.tile_pool(name="ps", bufs=8, space="PSUM") as pspool,
        ):
            wfq = [wpool.tile([128, 4 * 512], bf16, name=f"w_fi1_q{q}")
                   for q in range(4)]
            wt1 = wpool.tile([128, 4 * 128], bf16, name="w_ft1_t")
            wci = wpool.tile([128, 4, 4 * 128], f8, name="w_ci1_t")
            wct = wpool.tile([128, 4, 4 * 128], f8, name="w_ct1_t")
            wAV = wpool.tile([128, 4, 4 * 128], f8, name="w_AV_t")
            wAT = wpool.tile([128, 4, 4 * 128], f8, name="w_AT_t")
            wFV = wpool.tile([128, 4 * 512], bf16, name="w_FV_t")
            wFT = wpool.tile([128, 4 * 512], bf16, name="w_FT_t")
            bt = wpool.tile([128, 24], f32, name="bias_t")

            # ---- preamble: big DMAs in consumption order, split across the
            # two HWDGE trigger engines (sync, scalar).
            xt_cur = xpool.tile([128, NT], bf16, tag="xt", bufs=2, name="xt_0")
            nc.sync.dma_start(wt1[:], w_ft1_d[:])
            nc.sync.dma_start(xt_cur[:], tT_d[:, 0:NT])
            x_cur = [xpool.tile([128, 4 * NT], bf16, tag=f"xq{q}", name=f"xq_0_{q}")
                     for q in range(4)]
            for h in range(2):
                nc.sync.dma_start(x_cur[0][:, 2 * h * NT:2 * (h + 1) * NT],
                                  dram_x(0, 256 * h, 256 * (h + 1)))
            for q in range(1, 4):
                nc.sync.dma_start(x_cur[q][:], dram_x(0, 512 * q, 512 * (q + 1)))
            nc.sync.dma_start(wct[:], w_ct1_d[:].rearrange("p (c n) -> p c n", c=4))
            nc.sync.dma_start(wci[:], w_ci1_d[:].rearrange("p (c n) -> p c n", c=4))

            nc.scalar.dma_start(bt[:], bias_d[:])
            for h in range(2):
                nc.scalar.dma_start(wfq[0][:, 1024 * h:1024 * (h + 1)],
                                    w_fi1_d[:, 1024 * h:1024 * (h + 1)])
            for q in range(1, 4):
                nc.scalar.dma_start(wfq[q][:], w_fi1_d[:, 2048 * q:2048 * (q + 1)])
            nc.scalar.dma_start(wFV[:], w_FV_d[:])
            nc.scalar.dma_start(wAV[:], w_AV_d[:].rearrange("p (c n) -> p c n", c=4))
            nc.scalar.dma_start(wFT[:], w_FT_d[:])
            nc.scalar.dma_start(wAT[:], w_AT_d[:].rearrange("p (c n) -> p c n", c=4))

            def mm_dr(ps_ap, wtile3, kp, m, mov3, start, stop):
                nc.tensor.matmul(
                    ps_ap,
                    wtile3[:, 2 * kp:2 * kp + 2, m * 128:(m + 1) * 128],
                    mov3[:, 2 * kp:2 * kp + 2, :],
                    start=start, stop=stop, perf_mode=DR,
                )

            for n in range(ntiles):
                c0 = n * NT

                # ---- t_ = relu(t @ ft1.T + b): bf16 ----
                ps2 = [pspool.tile([128, NT], f32, tag="ps", name=f"ps2_{n}_{m}")
                       for m in range(4)]
                for m in range(4):
                    nc.tensor.matmul(ps2[m][:], wt1[:, m * 128:(m + 1) * 128],
                                     xt_cur[:], start=True, stop=True)
                t_b = [hpool.tile([128, NT], bf16, tag="t_", bufs=8,
                                  name=f"t_b_{n}_{m}") for m in range(4)]
                t_8 = hpool.tile([128, 4, NT], f8, tag="t8", name=f"t_8_{n}")
                for m in range(4):
                    nc.scalar.activation(t_b[m][:], ps2[m][:], Relu,
                                         bias=bt[:, 4 + m:5 + m])
                for m in range(4):
                    nc.vector.tensor_scalar_mul(t_8[:, m, :], t_b[m][:], S_T8)

                # ---- i_ = relu(i @ fi1.T + b): bf16 ----
                ps1 = [pspool.tile([128, NT], f32, tag="ps", name=f"ps1_{n}_{m}")
                       for m in range(4)]
                for k in range(16):
                    q, r = divmod(k, 4)
                    for m in range(4):
                        nc.tensor.matmul(
                            ps1[m][:],
                            wfq[q][:, 512 * r + 128 * m:512 * r + 128 * (m + 1)],
                            x_cur[q][:, NT * r:NT * (r + 1)],
                            start=k == 0, stop=k == 15)

                # prefetch next tile's inputs (single big DMAs)
                if n + 1 < ntiles:
                    x_nxt = [xpool.tile([128, 4 * NT], bf16, tag=f"xq{q}",
                                        name=f"xq_{n + 1}_{q}") for q in range(4)]
                    for q in range(4):
                        nc.sync.dma_start(
                            x_nxt[q][:], dram_x(c0 + NT, 512 * q, 512 * (q + 1)))
                    xt_nxt = xpool.tile([128, NT], bf16, tag="xt", bufs=2,
                                        name=f"xt_{n + 1}")
                    nc.sync.dma_start(xt_nxt[:], tT_d[:, c0 + NT:c0 + 2 * NT])

                i_b = [hpool.tile([128, NT], bf16, tag="i_", bufs=8,
                                  name=f"i_b_{n}_{m}") for m in range(4)]
                i_8 = hpool.tile([128, 4, NT], f8, tag="i8", name=f"i_8_{n}")
                for m in range(4):
                    nc.scalar.activation(i_b[m][:], ps1[m][:], Relu,
                                         bias=bt[:, m:m + 1])
                for m in range(4):
                    nc.vector.tensor_scalar_mul(i_8[:, m, :], i_b[m][:], S_I8)

                # ---- v2 = relu(t_ @ ct1.T + b): fp8 DoubleRow ----
                ps4 = [pspool.tile([128, NT], f32, tag="ps", name=f"ps4_{n}_{m}")
                       for m in range(4)]
                for kp in range(2):
                    for m in range(4):
                        mm_dr(ps4[m][:], wct, kp, m, t_8, kp == 0, kp == 1)
                v2_8 = hpool.tile([128, 4, NT], f8, tag="v2", name=f"v2_8_{n}")
                for m in range(4):
                    nc.scalar.activation(v2_8[:, m, :], ps4[m][:], Relu,
                                         bias=bt[:, 12 + m:13 + m], scale=g_v2)

                # ---- v1 = relu(i_ @ ci1.T + b): fp8 DoubleRow ----
                ps3 = [pspool.tile([128, NT], f32, tag="ps", name=f"ps3_{n}_{m}")
                       for m in range(4)]
                for kp in range(2):
                    for m in range(4):
                        mm_dr(ps3[m][:], wci, kp, m, i_8, kp == 0, kp == 1)
                v1_8 = hpool.tile([128, 4, NT], f8, tag="v1", name=f"v1_8_{n}")
                for m in range(4):
                    nc.scalar.activation(v1_8[:, m, :], ps3[m][:], Relu,
                                         bias=bt[:, 8 + m:9 + m], scale=g_v1)

                # ---- V = (v1 @ A.T)*sv + i_ @ (F*sv).T, then descale ----
                psV = [pspool.tile([128, NT], f32, tag="ps", name=f"psV_{n}_{m}")
                       for m in range(4)]
                for k in range(4):
                    for m in range(4):
                        nc.tensor.matmul(
                            psV[m][:],
                            wFV[:, 512 * k + 128 * m:512 * k + 128 * (m + 1)],
                            i_b[k][:], start=k == 0, stop=False)
                for kp in range(2):
                    for m in range(4):
                        mm_dr(psV[m][:], wAV, kp, m, v1_8, False, kp == 1)
                oV = opool.tile([128, 4, NT], bf16, tag="oV", name=f"oV_{n}")
                for m in range(4):
                    nc.vector.tensor_scalar(oV[:, m, :], psV[m][:], inv_sv,
                                            bt[:, 16 + m:17 + m], Mult, Add)
                nc.sync.dma_start(dram_out(0, c0), oV[:])

                # ---- T = (v2 @ A.T)*st + t_ @ (F*st).T, then descale ----
                psT = [pspool.tile([128, NT], f32, tag="ps", name=f"psT_{n}_{m}")
                       for m in range(4)]
                for k in range(4):
                    for m in range(4):
                        nc.tensor.matmul(
                            psT[m][:],
                            wFT[:, 512 * k + 128 * m:512 * k + 128 * (m + 1)],
                            t_b[k][:], start=k == 0, stop=False)
                if n + 1 == ntiles:
                    for m in range(4):
                        for kp in range(2):
                            mm_dr(psT[m][:], wAT, kp, m, v2_8, False, kp == 1)
                else:
                    for kp in range(2):
                        for m in range(4):
                            mm_dr(psT[m][:], wAT, kp, m, v2_8, False, kp == 1)
                if n + 1 == ntiles:
                    # drain the last tile m-block by m-block for a short tail
                    for m in range(4):
                        oT = opool.tile([128, NT], bf16, tag="oTl",
                                        bufs=4, name=f"oT_{n}_{m}")
                        nc.vector.tensor_scalar(oT[:], psT[m][:], inv_st,
                                                bt[:, 20 + m:21 + m], Mult, Add)
                        eng = nc.scalar if m % 2 else nc.sync
                        eng.dma_start(
                            out_d[512 + 128 * m:512 + 128 * (m + 1), c0:c0 + NT],
                            oT[:])
                else:
                    oT = opool.tile([128, 4, NT], bf16, tag="oT", name=f"oT_{n}")
                    for m in range(4):
                        nc.vector.tensor_scalar(oT[:, m, :], psT[m][:], inv_st,
                                                bt[:, 20 + m:21 + m], Mult, Add)
                    nc.sync.dma_start(dram_out(1, c0), oT[:])

                if n + 1 < ntiles:
                    x_cur = x_nxt
                    xt_cur = xt_nxt

    nc.compile()
    return nc


def _host_pack(inp: dict):
    f8d = np.float64
    bf = ml_dtypes.bfloat16
    e4 = ml_dtypes.float8_e4m3

    def fold(wv, bv, wo, bo, f_w, f_b):
        Wvo = wo.astype(f8d) @ wv.astype(f8d)
        bvo = wo.astype(f8d) @ bv.astype(f8d) + bo.astype(f8d)
        A = (f_w.astype(f8d) @ Wvo).astype(np.float32)
        F = f_w.astype(np.float32)
        bcat = (f_w.astype(f8d) @ bvo + f_b.astype(f8d)).astype(np.float32)
        return A, F, bcat

    AV, FV, bcatV = fold(inp["aV_wv"], inp["aV_bv"], inp["aV_wo"], inp["aV_bo"],
                         inp["fi2_w"], inp["fi2_b"])
    AT, FT, bcatT = fold(inp["aT_wv"], inp["aT_bv"], inp["aT_wo"], inp["aT_bo"],
                         inp["ft2_w"], inp["ft2_b"])

    s_wci = _pow2(160.0 / float(np.abs(inp["ci1_w"]).max()))
    s_wct = _pow2(160.0 / float(np.abs(inp["ct1_w"]).max()))
    s_AV = _pow2(160.0 / float(np.abs(AV).max()))
    s_AT = _pow2(160.0 / float(np.abs(AT).max()))
    sv = np.float32(S_V1 * s_AV)
    st = np.float32(S_V2 * s_AT)

    def q8(x, s):
        return np.clip(x * np.float32(s), -240, 240)

    def pack_f8(WT, s):
        if not SWI:
            return _pack_blocks(q8(WT, s), 4, 4, e4)
        W4 = q8(WT, s).reshape(4, 128, 4, 128)      # [ksub, p, m, j]
        arr = np.empty((128, 4, 4, 128), dtype=np.float32)  # [p, ksub, m, j]
        for kp in range(2):
            A = W4[2 * kp].transpose(0, 1, 2)        # [p, m, j]
            Bm = W4[2 * kp + 1]
            flat = np.empty((128, 4, 256), dtype=np.float32)
            flat[:, :, 0::2] = A[:, :, ::-1]
            flat[:, :, 1::2] = Bm[:, :, ::-1]
            arr[:, 2 * kp] = flat[:, :, 0:128]
            arr[:, 2 * kp + 1] = flat[:, :, 128:256]
        return np.ascontiguousarray(arr.reshape(128, 2048).astype(e4))

    tr = lambda w: np.ascontiguousarray(w.T)
    weights = {
        "w_fi1": _pack_blocks(tr(inp["fi1_w"]).astype(np.float32), 16, 4, bf),
        "w_ft1": _pack_blocks(tr(inp["ft1_w"]).astype(np.float32), 1, 4, bf),
        "w_ci1": pack_f8(tr(inp["ci1_w"]), s_wci),
        "w_ct1": pack_f8(tr(inp["ct1_w"]), s_wct),
        "w_AV": pack_f8(tr(AV), s_AV),
        "w_AT": pack_f8(tr(AT), s_AT),
        "w_FV": _pack_blocks(tr(FV) * sv, 4, 4, bf),
        "w_FT": _pack_blocks(tr(FT) * st, 4, 4, bf),
    }
    cols = []
    for b in (inp["fi1_b"], inp["ft1_b"],
              np.float32(S_V1) * inp["ci1_b"], np.float32(S_V2) * inp["ct1_b"],
              bcatV, bcatT):
        b = np.asarray(b, dtype=np.float32)
        for m in range(4):
            cols.append(b[128 * m:128 * (m + 1)])
    weights["bias"] = np.ascontiguousarray(np.stack(cols, axis=1),
                                           dtype=np.float32)
    scales = dict(
        g_v1=float(S_V1 / (S_I8 * s_wci)),
        g_v2=float(S_V2 / (S_T8 * s_wct)),
        inv_sv=float(1.0 / sv),
        inv_st=float(1.0 / st),
    )
    return weights, scales


def kernel(**inputs) -> np.ndarray:
    from concourse import bass_utils

    i = np.asarray(inputs["i"], dtype=np.float32)
    t = np.asarray(inputs["t"], dtype=np.float32)
    weights, scales = _host_pack(inputs)

    if "nc" not in _CACHE:
        _CACHE["nc"] = _build_nc(BS, **scales)
    nc = _CACHE["nc"]

    in_maps = []
    for c in range(NCORES):
        sl = slice(c * BS, (c + 1) * BS)
        m = dict(weights)
        m["iT"] = np.ascontiguousarray(i[sl].T.astype(ml_dtypes.bfloat16))
        m["tT"] = np.ascontiguousarray(t[sl].T.astype(ml_dtypes.bfloat16))
        in_maps.append(m)

    res = bass_utils.run_bass_kernel_spmd(nc, in_maps, core_ids=list(range(NCORES)))

    out = np.empty((B, 2 * HID), dtype=np.float32)
    for c in range(NCORES):
        out[c * BS:(c + 1) * BS] = res.results[c]["outT"].astype(np.float32).T
    return out
